# revision 1
# baseline (speedup 1.0000x reference)
"""Longformer (sliding-window attention) forward pass on 8 Trainium2 NeuronCores.

Sharding: sequence-parallel. 8 shards of 1024 tokens (4 shards per batch
element). Each core keeps a 32-token halo on each side of its shard; the halo
is refreshed after every layer with a boundary-block AllGather collective +
an indirect-DMA neighbor pick (per-core offsets are input data, so the SPMD
program stays identical across cores).

Device layout: activations are "d-major" ([d on partitions, token on free]) so
every matmul contracts over the partition dimension without transposes. V is
produced token-major straight from its projection matmul because probs@V
contracts over keys. Attention scores are computed key-major [g, t]; softmax
uses exp(x) without max-subtraction (scores are bounded: layernormed inputs,
~N(0, 0.02^2) weights), with additive -30 masking. The softmax denominator is
a ones-column matmul; normalization is a K=1 broadcast matmul + multiply fused
with the PSUM->SBUF copy.

Precision: bf16 matmul inputs / fp32 PSUM accumulation; residual stream bf16;
layernorm statistics fp32 (ones-matmuls); rsqrt via exp(-0.5*ln(var+eps)) on
the scalar engine (the ACT sqrt table is inaccurate, and ln/exp share one
table set with the attention exp).
"""

import os
import numpy as np
import ml_dtypes

import concourse.bass as bass
import concourse.bacc as bacc
import concourse.mybir as mybir
from concourse.tile import TileContext
from concourse.bass import IndirectOffsetOnAxis
from concourse.bass_utils import run_bass_kernel_spmd

FP32 = mybir.dt.float32
BF16 = mybir.dt.bfloat16
INT32 = mybir.dt.int32
ALU = mybir.AluOpType
AF = mybir.ActivationFunctionType
AX = mybir.AxisListType

# model dims
B, S, D, H, L_FULL, V, NCOUT = 2, 4096, 768, 12, 12, 50257, 16
DH = D // H            # 64
DFF = 4 * D            # 3072
W = 32                 # one-sided window
EPS = 1e-12
NC_CORES = 8
SHARDS_PER_B = 4
OWN = S // SHARDS_PER_B      # 1024 tokens per shard
EXT = OWN + 2 * W            # 1088 with halo
EXTP = 1152                  # EXT padded to 9*128 for the embedding gather
DK = D // 128                # 6 partition chunks of d
DFFK = DFF // 128            # 24 chunks of dff
NQC = OWN // 128             # 8 query chunks per shard
GW = 192                     # keys per 128-query chunk (128 + 2W + 64)
MASK_NEG = -30.0

L = int(os.environ.get("KERNEL_LAYERS", str(L_FULL)))


def build_nc(n_layers: int):
    nc = bacc.Bacc("TRN2", target_bir_lowering=False, debug=False,
                   num_devices=NC_CORES)

    # ---------------- DRAM I/O ----------------
    emb_d = nc.dram_tensor("emb", [V, D], BF16, kind="ExternalInput")
    xids_d = nc.dram_tensor("xids", [EXTP // 128, 128], INT32, kind="ExternalInput")
    pos_d = nc.dram_tensor("postok", [EXTP, D], FP32, kind="ExternalInput")
    maskP_d = nc.dram_tensor("maskP", [NQC, 128, 512], BF16, kind="ExternalInput")
    hofs_d = nc.dram_tensor("hofs", [2 * DK, 128], INT32, kind="ExternalInput")
    ident_d = nc.dram_tensor("ident", [128, 128], BF16, kind="ExternalInput")
    wq_d = nc.dram_tensor("wq", [n_layers, D, D], BF16, kind="ExternalInput")
    wk_d = nc.dram_tensor("wk", [n_layers, D, D], BF16, kind="ExternalInput")
    wv_d = nc.dram_tensor("wv", [n_layers, D, D], BF16, kind="ExternalInput")
    wo_d = nc.dram_tensor("wo", [n_layers, D, D], BF16, kind="ExternalInput")
    w1_d = nc.dram_tensor("w1", [n_layers, D, DFF], BF16, kind="ExternalInput")
    w2_d = nc.dram_tensor("w2", [n_layers, DFF, D], BF16, kind="ExternalInput")
    fcw_d = nc.dram_tensor("fcw", [D, NCOUT], FP32, kind="ExternalInput")
    out_d = nc.dram_tensor("out", [NCOUT, 1], FP32, kind="ExternalOutput")

    # per-layer collective bounce buffers (internal DRAM)
    ag_in = [nc.dram_tensor(f"ag_in_{l}", [D, 2, W], BF16)
             for l in range(n_layers - 1)]
    ag_out = [nc.dram_tensor(f"ag_out_{l}", [NC_CORES, D, 2, W], BF16,
                             addr_space="Shared")
              for l in range(n_layers - 1)]

    wview = {}
    for name, t in (("wq", wq_d), ("wk", wk_d), ("wv", wv_d),
                    ("wo", wo_d), ("w1", w1_d), ("w2", w2_d)):
        wview[name] = t.ap().rearrange("l (a p) n -> l p a n", p=128)

    with TileContext(nc) as tc:
        with (
            tc.tile_pool(name="const", bufs=1) as cpool,
            tc.tile_pool(name="hpool", bufs=2) as hpool,
            tc.tile_pool(name="big", bufs=1) as bpool,
            tc.tile_pool(name="stream", bufs=3) as spool,
            tc.tile_pool(name="small", bufs=3) as smpool,
            tc.tile_pool(name="psum", bufs=2, space="PSUM") as ppool,
        ):
            pools = (hpool, bpool, spool, smpool, ppool)
            # ---------------- constants ----------------
            ones_col = cpool.tile([128, 1], BF16, tag="ones_col")
            nc.vector.memset(ones_col[:], 1.0)
            ones_row = cpool.tile([1, 128], BF16, tag="ones_row")
            nc.vector.memset(ones_row[:], 1.0)
            cneg_row = cpool.tile([1, 128], BF16, tag="cneg_row")
            nc.vector.memset(cneg_row[:], -1.0 / D)
            eps_col = cpool.tile([128, 1], FP32, tag="eps_col")
            nc.vector.memset(eps_col[:], EPS)
            ident = cpool.tile([128, 128], BF16, tag="ident")
            nc.sync.dma_start(ident[:], ident_d[:, :])

            offs = cpool.tile([128, EXTP // 128], INT32, tag="offs")
            nc.sync.dma_start(offs[:], xids_d.ap().rearrange("a p -> p a"))
            hofs = cpool.tile([128, 2 * DK], INT32, tag="hofs")
            nc.sync.dma_start(hofs[:], hofs_d.ap().rearrange("a p -> p a"))
            maskP = cpool.tile([128, NQC, 512], BF16, tag="maskP")
            nc.sync.dma_start(maskP[:], maskP_d.ap().rearrange("a g t -> g a t"))
            fcw = cpool.tile([128, DK, NCOUT], FP32, tag="fcw")
            nc.sync.dma_start(fcw[:], fcw_d.ap().rearrange("(a p) n -> p a n", p=128))
            consts = (ones_col, ones_row, cneg_row, ident, offs, hofs,
                      maskP, eps_col)

            # ---------------- embedding + LN (token-major) ----------------
            h = hpool.tile([128, DK, EXT], BF16, tag="h")
            for c in range(EXTP // 128):
                emb_tm = spool.tile([128, D], BF16, tag="emb_tm", bufs=2)
                nc.gpsimd.indirect_dma_start(
                    out=emb_tm[:], out_offset=None, in_=emb_d[:, :],
                    in_offset=IndirectOffsetOnAxis(ap=offs[:, c:c + 1], axis=0),
                )
                pos_sb = spool.tile([128, D], FP32, tag="pos_sb", bufs=2)
                nc.sync.dma_start(pos_sb[:], pos_d[c * 128:(c + 1) * 128, :])
                x0 = spool.tile([128, D], FP32, tag="x0", bufs=2)
                nc.vector.tensor_tensor(out=x0[:], in0=emb_tm[:], in1=pos_sb[:],
                                        op=ALU.add)
                st6 = smpool.tile([128, 2, 6], FP32, tag="st6")
                nc.vector.bn_stats(st6[:, 0, :], x0[:, 0:384])
                nc.vector.bn_stats(st6[:, 1, :], x0[:, 384:768])
                agg = smpool.tile([128, 2], FP32, tag="agg")
                nc.vector.bn_aggr(agg[:], st6[:].rearrange("p a b -> p (a b)"))
                lnv = smpool.tile([128, 1], FP32, tag="lnv")
                nc.scalar.activation(lnv[:], agg[:, 1:2], AF.Ln, bias=eps_col[:])
                rstd = smpool.tile([128, 1], FP32, tag="rstd")
                nc.scalar.activation(rstd[:], lnv[:], AF.Exp, scale=-0.5)
                hn_tm = spool.tile([128, D], BF16, tag="hn_tm", bufs=2)
                nc.vector.tensor_scalar(
                    out=hn_tm[:], in0=x0[:], scalar1=agg[:, 0:1],
                    scalar2=rstd[:], op0=ALU.subtract, op1=ALU.mult)
                # transpose to d-major
                ncols = min(128, EXT - c * 128)
                for k in range(DK):
                    ps_t = ppool.tile([128, 128], BF16, tag="p0", bufs=1, name="ps_t")
                    nc.tensor.transpose(ps_t[:], hn_tm[:, k * 128:(k + 1) * 128],
                                        ident[:])
                    nc.vector.tensor_copy(
                        out=h[:, k, c * 128:c * 128 + ncols],
                        in_=ps_t[:, :ncols])

            # ---------------- layers ----------------
            for l in range(n_layers):
                h = layer(nc, l, h, wview, consts, ag_in, ag_out, n_layers,
                          pools)

            # ---------------- final mean + fc ----------------
            hsum = smpool.tile([128, DK], FP32, tag="hsum")
            for k in range(DK):
                nc.vector.tensor_reduce(out=hsum[:, k:k + 1],
                                        in_=h[:, k, W:W + OWN],
                                        axis=AX.X, op=ALU.add)
            ps_fc = ppool.tile([NCOUT, 1], FP32, tag="p0", bufs=1, name="ps_fc")
            for k in range(DK):
                nc.tensor.matmul(ps_fc[:], fcw[:, k, :], hsum[:, k:k + 1],
                                 start=(k == 0), stop=(k == DK - 1))
            out_sb = smpool.tile([NCOUT, 1], FP32, tag="out_sb")
            nc.vector.tensor_copy(out_sb[:], ps_fc[:])
            nc.sync.dma_start(out_d[:, :], out_sb[:])

    nc.compile()
    return nc


def ln_d_major(nc, src, dst, dst_off, tcs, consts, spool, smpool, ppool, tag):
    """Layernorm over d for d-major bf16 activations.

    src: [128, DK, ntok] AP; result written to dst[:, k, dst_off + t].
    tcs: list of (t_start, t_len) chunks (<=512).
    """
    ones_col, ones_row, cneg_row = consts[0], consts[1], consts[2]
    eps_row = consts[7][0:1, :]
    for (t0, tl) in tcs:
        sum_ps = ppool.tile([1, 512], FP32, tag="p2", bufs=1, name="sum_ps")
        sq_ps = ppool.tile([1, 512], FP32, tag="p3", bufs=1, name="sq_ps")
        for k in range(DK):
            sqt = spool.tile([128, 512], BF16, tag="sqt")
            nc.scalar.square(sqt[:, :tl], src[:, k, t0:t0 + tl])
            nc.tensor.matmul(sum_ps[:, :tl], ones_col[:], src[:, k, t0:t0 + tl],
                             start=(k == 0), stop=(k == DK - 1))
            nc.tensor.matmul(sq_ps[:, :tl], ones_col[:], sqt[:, :tl],
                             start=(k == 0), stop=(k == DK - 1))
        def row(nm):
            return smpool.tile([1, 512], FP32, tag="lnrow", bufs=5, name=nm)
        sum_sb = row("sum_sb")
        nc.vector.tensor_copy(sum_sb[:, :tl], sum_ps[:, :tl])
        t1 = row("t1")
        nc.vector.tensor_tensor(out=t1[:, :tl], in0=sum_sb[:, :tl],
                                in1=sum_sb[:, :tl], op=ALU.mult)
        t2 = row("t2")
        nc.vector.tensor_scalar(out=t2[:, :tl], in0=t1[:, :tl],
                                scalar1=-1.0 / D, scalar2=None, op0=ALU.mult)
        diff = row("diff")
        nc.vector.tensor_tensor(out=diff[:, :tl], in0=sq_ps[:, :tl],
                                in1=t2[:, :tl], op=ALU.add)
        dpos = row("dpos")
        nc.vector.tensor_scalar(out=dpos[:, :tl], in0=diff[:, :tl],
                                scalar1=0.0, scalar2=None, op0=ALU.max)
        lnv = row("lnv")
        nc.scalar.activation(lnv[:, :tl], dpos[:, :tl], AF.Ln,
                             bias=eps_row, scale=1.0 / D)
        rstd = row("rstd")
        nc.scalar.activation(rstd[:, :tl], lnv[:, :tl], AF.Exp, scale=-0.5)
        mr = row("mr")
        nc.vector.tensor_tensor(out=mr[:, :tl], in0=sum_sb[:, :tl],
                                in1=rstd[:, :tl], op=ALU.mult)
        r_bf = smpool.tile([1, 512], BF16, tag="lnrow_bf", bufs=2, name="r_bf")
        nc.vector.tensor_copy(r_bf[:, :tl], rstd[:, :tl])
        mr_bf = smpool.tile([1, 512], BF16, tag="lnrow_bf", bufs=2,
                            name="mr_bf")
        nc.vector.tensor_copy(mr_bf[:, :tl], mr[:, :tl])
        rb_ps = ppool.tile([128, 512], FP32, tag="p4", bufs=1, name="rb_ps")
        nc.tensor.matmul(rb_ps[:, :tl], ones_row[:], r_bf[:, :tl],
                         start=True, stop=True)
        mrb_ps = ppool.tile([128, 512], FP32, tag="p5", bufs=1, name="mrb_ps")
        nc.tensor.matmul(mrb_ps[:, :tl], cneg_row[:], mr_bf[:, :tl],
                         start=True, stop=True)
        rbs = spool.tile([128, 512], BF16, tag="rbs", bufs=2, name="rbs")
        nc.scalar.copy(rbs[:, :tl], rb_ps[:, :tl])
        mrbs = spool.tile([128, 512], BF16, tag="mrbs", bufs=2, name="mrbs")
        nc.scalar.copy(mrbs[:, :tl], mrb_ps[:, :tl])
        for k in range(DK):
            tmp = spool.tile([128, 512], BF16, tag="lnap")
            nc.vector.tensor_tensor(out=tmp[:, :tl], in0=src[:, k, t0:t0 + tl],
                                    in1=rbs[:, :tl], op=ALU.mult)
            nc.vector.tensor_tensor(out=dst[:, k, dst_off + t0:dst_off + t0 + tl],
                                    in0=tmp[:, :tl], in1=mrbs[:, :tl],
                                    op=ALU.add)


def layer(nc, l, h, wview, consts, ag_in, ag_out, n_layers, pools):
    hpool, bpool, spool, smpool, ppool = pools
    (ones_col, ones_row, cneg_row, ident, offs, hofs, maskP,
     eps_col) = consts
    TC_EXT = [(0, 512), (512, 512), (1024, EXT - 1024)]
    TC_OWN = [(0, 512), (512, 512)]

    # ---------------- QKV ----------------
    mmctr = [0]

    def mm_tile(name):
        t = ppool.tile([128, 512], FP32, tag=f"p{mmctr[0] % 2}", bufs=1,
                       name=name)
        mmctr[0] += 1
        return t

    q_sb = bpool.tile([128, DK, OWN], BF16, tag="q")
    k_sb = bpool.tile([128, DK, EXT], BF16, tag="k")
    for name, dst in (("wq", q_sb), ("wk", k_sb)):
        is_q = name == "wq"
        for ko in range(DK):
            wt = spool.tile([128, DK, 128], BF16, tag="wt")
            nc.sync.dma_start(wt[:], wview[name][l, :, :, ko * 128:(ko + 1) * 128])
            tcs = TC_OWN if is_q else TC_EXT
            off = W if is_q else 0
            for (t0, tl) in tcs:
                ps = mm_tile("ps_qk")
                for ki in range(DK):
                    nc.tensor.matmul(ps[:, :tl], wt[:, ki, :],
                                     h[:, ki, off + t0:off + t0 + tl],
                                     start=(ki == 0), stop=(ki == DK - 1))
                nc.vector.tensor_copy(dst[:, ko, t0:t0 + tl], ps[:, :tl])
    # V: token-major [token, d]
    v_tm = bpool.tile([128, 9, D], BF16, tag="v")
    wv_t = bpool.tile([128, DK, D], BF16, tag="wv_full")
    nc.sync.dma_start(wv_t[:], wview["wv"][l])
    for c in range(9):
        ncols = min(128, EXT - c * 128)
        for d0, dl in ((0, 512), (512, 256)):
            ps = mm_tile("ps_v")
            for ki in range(DK):
                nc.tensor.matmul(ps[:ncols, :dl],
                                 h[:, ki, c * 128:c * 128 + ncols],
                                 wv_t[:, ki, d0:d0 + dl],
                                 start=(ki == 0), stop=(ki == DK - 1))
            nc.scalar.copy(v_tm[:ncols, c, d0:d0 + dl], ps[:ncols, :dl])

    # ---------------- attention (head pairs) ----------------
    a_sb = bpool.tile([128, DK, OWN], BF16, tag="attn")
    for qc in range(NQC):
        for j in range(H // 2):
            par = (qc * (H // 2) + j) % 3
            g0 = qc * 128
            # one PSUM bank holds both heads' scores: [evA|odA|evB|odB]
            sc = ppool.tile([128, 512], FP32, tag=f"p{2 + 2 * par}", bufs=1,
                            name="sc")
            nc.tensor.matmul(sc[:], ident[:], maskP[:, qc, :],
                             start=True, stop=False)
            for par_h, ro in ((0, 0), (1, 64)):
                q_ap = q_sb[ro:ro + 64, j, g0:g0 + 128]
                nc.tensor.matmul(sc[:, 128 * par_h:128 * par_h + 128],
                                 k_sb[ro:ro + 64, j, g0:g0 + 128], q_ap,
                                 start=False, stop=False, skip_group_check=True)
                nc.tensor.matmul(sc[:64, 256 + 128 * par_h:384 + 128 * par_h],
                                 k_sb[ro:ro + 64, j, g0 + 128:g0 + GW], q_ap,
                                 start=False, stop=(par_h == 1),
                                 skip_group_check=True)
            eA = spool.tile([128, 256], BF16, tag="eA")
            nc.scalar.activation(eA[:], sc[:, 0:256], AF.Exp)
            eB = spool.tile([64, 256], BF16, tag="eB")
            nc.scalar.activation(eB[:], sc[:64, 256:512], AF.Exp)
            pvse = ppool.tile([128, 512], FP32, tag=f"p{3 + 2 * par}", bufs=1,
                              name="pvse")
            se = pvse[0:1, 256:512]
            nc.tensor.matmul(se, ones_col[:], eA[:], start=True, stop=False)
            nc.tensor.matmul(se, ones_col[:64, :], eB[:], start=False,
                             stop=True)
            for par_h, po in ((0, 0), (1, 64)):
                pv = pvse[po:po + 64, 128 * par_h:128 * par_h + 128]
                nc.tensor.matmul(
                    pv, v_tm[:, qc, 128 * j + 64 * par_h:128 * j + 64 * par_h + 64],
                    eA[:, 128 * par_h:128 * par_h + 128], start=True, stop=False,
                    tile_position=(0, po), skip_group_check=True)
                nc.tensor.matmul(
                    pv, v_tm[:64, qc + 1, 128 * j + 64 * par_h:128 * j + 64 * par_h + 64],
                    eB[:, 128 * par_h:128 * par_h + 128], start=False, stop=True,
                    tile_position=(0, po), skip_group_check=True)
            rcp_bf = smpool.tile([1, 256], BF16, tag="rcp_bf")
            with nc.allow_low_precision("softmax denominator"):
                nc.vector.reciprocal(rcp_bf[:], se)
            rb = pvse[:, 256:512]
            nc.tensor.matmul(rb, ones_row[:], rcp_bf[:], start=True,
                             stop=True, skip_group_check=True)
            rb_sb = spool.tile([128, 256], BF16, tag="rb_sb")
            nc.vector.tensor_copy(rb_sb[:], rb)
            for par_h, po in ((0, 0), (1, 64)):
                nc.vector.tensor_tensor(
                    out=a_sb[po:po + 64, j, g0:g0 + 128],
                    in0=pvse[po:po + 64, 128 * par_h:128 * par_h + 128],
                    in1=rb_sb[po:po + 64, 128 * par_h:128 * par_h + 128],
                    op=ALU.mult)

    # ---------------- Wo + residual -> LN1 -> h2 ----------------
    h2raw = bpool.tile([128, DK, OWN], BF16, tag="h2raw")
    for ko in range(DK):
        wt = spool.tile([128, DK, 128], BF16, tag="wt")
        nc.sync.dma_start(wt[:], wview["wo"][l, :, :, ko * 128:(ko + 1) * 128])
        for (t0, tl) in TC_OWN:
            ps = mm_tile("ps_wo")
            for ki in range(DK):
                nc.tensor.matmul(ps[:, :tl], wt[:, ki, :], a_sb[:, ki, t0:t0 + tl],
                                 start=(ki == 0), stop=(ki == DK - 1))
            nc.vector.tensor_tensor(out=h2raw[:, ko, t0:t0 + tl],
                                    in0=h[:, ko, W + t0:W + t0 + tl],
                                    in1=ps[:, :tl], op=ALU.add)
    h2 = bpool.tile([128, DK, OWN], BF16, tag="h2")
    ln_d_major(nc, h2raw, h2, 0, TC_OWN, consts, spool, smpool, ppool, "ln1")

    # ---------------- FFN -> residual -> LN2 -> h3 ----------------
    h3raw = bpool.tile([128, DK, OWN], BF16, tag="h3raw")
    for (t0, tl) in TC_OWN:
        accs = [ppool.tile([128, 512], FP32, tag=f"p{j + 2}", bufs=1,
                           name=f"acc{j}") for j in range(DK)]
        for j in range(DFFK):
            w1t = spool.tile([128, DK, 128], BF16, tag="w1t")
            nc.sync.dma_start(w1t[:], wview["w1"][l, :, :, j * 128:(j + 1) * 128])
            w2t = spool.tile([128, D], BF16, tag="w2t")
            nc.sync.dma_start(w2t[:], wview["w2"][l, :, j, :])
            ps1 = ppool.tile([128, 512], FP32, tag=f"p{j % 2}", bufs=1,
                             name="ps1")
            for ki in range(DK):
                nc.tensor.matmul(ps1[:, :tl], w1t[:, ki, :], h2[:, ki, t0:t0 + tl],
                                 start=(ki == 0), stop=(ki == DK - 1))
            g = spool.tile([128, 512], BF16, tag="gel")
            nc.scalar.activation(g[:, :tl], ps1[:, :tl], AF.Gelu)
            for ko in range(DK):
                nc.tensor.matmul(accs[ko][:, :tl], w2t[:, ko * 128:(ko + 1) * 128],
                                 g[:, :tl], start=(j == 0), stop=(j == DFFK - 1))
        for ko in range(DK):
            nc.vector.tensor_tensor(out=h3raw[:, ko, t0:t0 + tl],
                                    in0=h2[:, ko, t0:t0 + tl],
                                    in1=accs[ko][:, :tl], op=ALU.add)
    h3 = hpool.tile([128, DK, EXT], BF16, tag="h")
    ln_d_major(nc, h3raw, h3, W, TC_OWN, consts, spool, smpool, ppool, "ln2")

    # ---------------- halo exchange ----------------
    if l < n_layers - 1:
        agi = ag_in[l].ap().rearrange("(a p) s c -> p a s c", p=128)
        nc.sync.dma_start(agi[:, :, 0, :], h3[:, :, W:2 * W])
        nc.sync.dma_start(agi[:, :, 1, :], h3[:, :, OWN:W + OWN])
        nc.gpsimd.collective_compute(
            "AllGather", ALU.bypass, replica_groups=[list(range(NC_CORES))],
            ins=[ag_in[l].ap()], outs=[ag_out[l].ap()],
        )
        agv = ag_out[l].ap().rearrange("s d b c -> (s d b) c")
        for k in range(DK):
            nc.gpsimd.indirect_dma_start(
                out=h3[:, k, 0:W], out_offset=None, in_=agv[:, :],
                in_offset=IndirectOffsetOnAxis(ap=hofs[:, k:k + 1], axis=0),
            )
            nc.gpsimd.indirect_dma_start(
                out=h3[:, k, W + OWN:EXT], out_offset=None, in_=agv[:, :],
                in_offset=IndirectOffsetOnAxis(ap=hofs[:, DK + k:DK + k + 1],
                                               axis=0),
            )
    return h3


_NC_CACHE = {}


def _get_nc(n_layers):
    if n_layers not in _NC_CACHE:
        _NC_CACHE[n_layers] = build_nc(n_layers)
    return _NC_CACHE[n_layers]


def make_in_maps(x, emb, pos_emb, tok_emb, Wq, Wk, Wv, Wo, W1, W2, fc_w,
                 n_layers):
    x = np.asarray(x)
    bf = lambda a: np.ascontiguousarray(np.asarray(a), dtype=np.float32).astype(
        ml_dtypes.bfloat16)
    scale = 1.0 / np.sqrt(np.float32(DH))
    shared = {
        "emb": bf(emb),
        "wq": bf(np.asarray(Wq)[:n_layers] * scale),
        "wk": bf(np.asarray(Wk)[:n_layers]),
        "wv": bf(np.asarray(Wv)[:n_layers]),
        "wo": bf(np.asarray(Wo)[:n_layers]),
        "w1": bf(np.asarray(W1)[:n_layers]),
        "w2": bf(np.asarray(W2)[:n_layers]),
        "fcw": np.ascontiguousarray(np.asarray(fc_w), dtype=np.float32),
        "ident": np.eye(128, dtype=ml_dtypes.bfloat16),
    }
    postok_full = (np.asarray(pos_emb)[1:S + 1] + np.asarray(tok_emb)[0]
                   ).astype(np.float32)

    in_maps = []
    for c in range(NC_CORES):
        b, s_idx = divmod(c, SHARDS_PER_B)
        own0 = s_idx * OWN
        ext_pos = np.arange(own0 - W, own0 - W + EXTP)
        valid = (ext_pos >= 0) & (ext_pos < S)
        pos_c = np.clip(ext_pos, 0, S - 1)
        xids = np.where(valid, x[b][pos_c], 1).astype(np.int32)
        postok = np.where(valid[:, None], postok_full[pos_c], 0.0
                          ).astype(np.float32)
        # masks: scores[g, t] for query chunk qc; key global position is
        # own0 - W + qc*128 + g, query global position own0 + qc*128 + t.
        gi = np.arange(GW)[:, None]
        ti = np.arange(128)[None, :]
        band = np.abs((gi - W) - ti) <= W
        mP = np.zeros((NQC, 128, 512), ml_dtypes.bfloat16)
        for qc in range(NQC):
            kpos = own0 - W + qc * 128 + np.arange(GW)
            ok = band & ((kpos >= 0) & (kpos < S))[:, None]
            m = np.where(ok, 0.0, MASK_NEG).astype(ml_dtypes.bfloat16)
            mP[qc, :, 0:128] = m[:128]
            mP[qc, :, 128:256] = m[:128]
            mP[qc, :64, 256:384] = m[128:]
            mP[qc, :64, 384:512] = m[128:]
        # halo gather offsets into ag_out viewed as rows [(8*768*2), 32]
        slot_l, slot_r = max(c - 1, 0), min(c + 1, NC_CORES - 1)
        p = np.arange(128)
        hofs = np.empty((2 * DK, 128), np.int32)
        for k in range(DK):
            hofs[k] = (slot_l * D + k * 128 + p) * 2 + 1
            hofs[DK + k] = (slot_r * D + k * 128 + p) * 2 + 0
        in_maps.append({
            **shared, "xids": xids.reshape(EXTP // 128, 128),
            "postok": postok, "maskP": mP, "hofs": hofs,
        })
    return in_maps


def kernel(x, emb, pos_emb, tok_emb, emb_ln_s, emb_ln_b, Wq, bq, Wk, bk,
           Wv, bv, Wo, bo, ln1_s, ln1_b, W1, b1, W2, b2, ln2_s, ln2_b,
           fc_w, fc_b, _n_layers=None, _results_hook=None):
    n_layers = _n_layers if _n_layers is not None else L
    for z in (bq, bk, bv, bo, b1, b2, emb_ln_b, ln1_b, ln2_b):
        assert not np.any(np.asarray(z)), "nonzero biases unsupported"
    for o in (emb_ln_s, ln1_s, ln2_s):
        assert np.all(np.asarray(o) == 1.0), "non-unit LN scales unsupported"

    in_maps = make_in_maps(x, emb, pos_emb, tok_emb, Wq, Wk, Wv, Wo, W1, W2,
                           fc_w, n_layers)
    nc = _get_nc(n_layers)
    res = run_bass_kernel_spmd(nc, in_maps, list(range(NC_CORES)))
    if _results_hook is not None:
        _results_hook(res)
    out = np.zeros((B, NCOUT), np.float32)
    for c in range(NC_CORES):
        out[c // SHARDS_PER_B] += res.results[c]["out"][:, 0]
    out = out / np.float32(S) + np.asarray(fc_b, np.float32)
    return out



# revision 9
# speedup vs baseline: 13.8343x; 13.8343x over previous
"""Longformer (sliding-window attention) forward pass on 8 Trainium2 NeuronCores.

Sharding: sequence-parallel. 8 shards of 1024 tokens (4 shards per batch
element). Each core keeps a 32-token halo on each side of its shard; the halo
is refreshed after every layer with a boundary-block AllGather collective +
an indirect-DMA neighbor pick (per-core offsets are input data, so the SPMD
program stays identical across cores).

Device layout: activations are "d-major" ([d on partitions, token on free]) so
every matmul contracts over the partition dimension without transposes. V is
produced token-major straight from its projection matmul because probs@V
contracts over keys. Attention scores are computed key-major [g, t]; softmax
uses exp(x) without max-subtraction (scores are bounded: layernormed inputs,
~N(0, 0.02^2) weights), with additive -30 masking. The softmax denominator is
a ones-column matmul; normalization is a K=1 broadcast matmul + multiply fused
with the PSUM->SBUF copy.

Precision: bf16 matmul inputs / fp32 PSUM accumulation; residual stream bf16;
layernorm statistics fp32 (ones-matmuls); rsqrt via exp(-0.5*ln(var+eps)) on
the scalar engine (the ACT sqrt table is inaccurate, and ln/exp share one
table set with the attention exp).
"""

import os
import numpy as np
import ml_dtypes

import concourse.bass as bass
import concourse.bacc as bacc
import concourse.mybir as mybir
from concourse.tile import TileContext
from concourse.bass import IndirectOffsetOnAxis
from concourse.bass_utils import run_bass_kernel_spmd

FP32 = mybir.dt.float32
BF16 = mybir.dt.bfloat16
INT32 = mybir.dt.int32
ALU = mybir.AluOpType
AF = mybir.ActivationFunctionType
AX = mybir.AxisListType

# model dims
B, S, D, H, L_FULL, V, NCOUT = 2, 4096, 768, 12, 12, 50257, 16
DH = D // H            # 64
DFF = 4 * D            # 3072
W = 32                 # one-sided window
EPS = 1e-12
NC_CORES = 8
SHARDS_PER_B = 4
OWN = S // SHARDS_PER_B      # 1024 tokens per shard
EXT = OWN + 2 * W            # 1088 with halo
EXTP = 1152                  # EXT padded to 9*128 for the embedding gather
DK = D // 128                # 6 partition chunks of d
DFFK = DFF // 128            # 24 chunks of dff
NQC = OWN // 128             # 8 query chunks per shard
GW = 192                     # keys per 128-query chunk (128 + 2W + 64)
MASK_NEG = -30.0

L = int(os.environ.get("KERNEL_LAYERS", str(L_FULL)))


def build_nc(n_layers: int):
    nc = bacc.Bacc("TRN2", target_bir_lowering=False, debug=False,
                   num_devices=NC_CORES)

    # ---------------- DRAM I/O ----------------
    emb_d = nc.dram_tensor("emb", [V, D], BF16, kind="ExternalInput")
    xids_d = nc.dram_tensor("xids", [EXTP // 128, 128], INT32, kind="ExternalInput")
    pos_d = nc.dram_tensor("postok", [EXTP, D], FP32, kind="ExternalInput")
    maskP_d = nc.dram_tensor("maskP", [NQC, 128, 512], BF16, kind="ExternalInput")
    hofs_d = nc.dram_tensor("hofs", [2 * DK, 128], INT32, kind="ExternalInput")
    ident_d = nc.dram_tensor("ident", [128, 128], BF16, kind="ExternalInput")
    wq_d = nc.dram_tensor("wq", [n_layers, D, D], BF16, kind="ExternalInput")
    wk_d = nc.dram_tensor("wk", [n_layers, D, D], BF16, kind="ExternalInput")
    wv_d = nc.dram_tensor("wv", [n_layers, D, D], BF16, kind="ExternalInput")
    wo_d = nc.dram_tensor("wo", [n_layers, D, D], BF16, kind="ExternalInput")
    w1_d = nc.dram_tensor("w1", [n_layers, D, DFF], BF16, kind="ExternalInput")
    w2_d = nc.dram_tensor("w2", [n_layers, DFF, D], BF16, kind="ExternalInput")
    fcw_d = nc.dram_tensor("fcw", [D, NCOUT], FP32, kind="ExternalInput")
    out_d = nc.dram_tensor("out", [NCOUT, 1], FP32, kind="ExternalOutput")

    # per-layer collective bounce buffers (internal DRAM)
    ag_in = [nc.dram_tensor(f"ag_in_{l}", [D, 2, W], BF16)
             for l in range(n_layers - 1)]
    ag_out = [nc.dram_tensor(f"ag_out_{l}", [NC_CORES, D, 2, W], BF16,
                             addr_space="Shared")
              for l in range(n_layers - 1)]

    wview = {}
    for name, t in (("wq", wq_d), ("wk", wk_d), ("wv", wv_d),
                    ("wo", wo_d), ("w1", w1_d), ("w2", w2_d)):
        wview[name] = t.ap().rearrange("l (a p) n -> l p a n", p=128)

    with TileContext(nc) as tc:
        with (
            tc.tile_pool(name="const", bufs=1) as cpool,
            tc.tile_pool(name="hpool", bufs=2) as hpool,
            tc.tile_pool(name="big", bufs=1) as bpool,
            tc.tile_pool(name="stream", bufs=3) as spool,
            tc.tile_pool(name="small", bufs=3) as smpool,
            tc.tile_pool(name="psum", bufs=2, space="PSUM") as ppool,
        ):
            pools = (hpool, bpool, spool, smpool, ppool)
            # ---------------- constants ----------------
            ones_col = cpool.tile([128, 1], BF16, tag="ones_col")
            nc.vector.memset(ones_col[:], 1.0)
            ones_row = cpool.tile([1, 128], BF16, tag="ones_row")
            nc.vector.memset(ones_row[:], 1.0)
            cneg_row = cpool.tile([1, 128], BF16, tag="cneg_row")
            nc.vector.memset(cneg_row[:], -1.0 / D)
            eps_col = cpool.tile([128, 1], FP32, tag="eps_col")
            nc.vector.memset(eps_col[:], EPS)
            ident = cpool.tile([128, 128], BF16, tag="ident")
            nc.sync.dma_start(ident[:], ident_d[:, :])

            offs = cpool.tile([128, EXTP // 128], INT32, tag="offs")
            nc.sync.dma_start(offs[:], xids_d.ap().rearrange("a p -> p a"))
            hofs = cpool.tile([128, 2 * DK], INT32, tag="hofs")
            nc.sync.dma_start(hofs[:], hofs_d.ap().rearrange("a p -> p a"))
            maskP = cpool.tile([128, NQC, 512], BF16, tag="maskP")
            nc.sync.dma_start(maskP[:], maskP_d.ap().rearrange("a g t -> g a t"))
            fcw = cpool.tile([128, DK, NCOUT], FP32, tag="fcw")
            nc.sync.dma_start(fcw[:], fcw_d.ap().rearrange("(a p) n -> p a n", p=128))
            consts = (ones_col, ones_row, cneg_row, ident, offs, hofs,
                      maskP, eps_col)

            # ---------------- embedding + LN (token-major) ----------------
            h = hpool.tile([128, DK, EXT], BF16, tag="h")
            for c in range(EXTP // 128):
                emb_tm = spool.tile([128, D], BF16, tag="emb_tm", bufs=2)
                nc.gpsimd.indirect_dma_start(
                    out=emb_tm[:], out_offset=None, in_=emb_d[:, :],
                    in_offset=IndirectOffsetOnAxis(ap=offs[:, c:c + 1], axis=0),
                )
                pos_sb = spool.tile([128, D], FP32, tag="pos_sb", bufs=2)
                nc.sync.dma_start(pos_sb[:], pos_d[c * 128:(c + 1) * 128, :])
                x0 = spool.tile([128, D], FP32, tag="x0", bufs=2)
                nc.vector.tensor_tensor(out=x0[:], in0=emb_tm[:], in1=pos_sb[:],
                                        op=ALU.add)
                st6 = smpool.tile([128, 2, 6], FP32, tag="st6")
                nc.vector.bn_stats(st6[:, 0, :], x0[:, 0:384])
                nc.vector.bn_stats(st6[:, 1, :], x0[:, 384:768])
                agg = smpool.tile([128, 2], FP32, tag="agg")
                nc.vector.bn_aggr(agg[:], st6[:].rearrange("p a b -> p (a b)"))
                lnv = smpool.tile([128, 1], FP32, tag="lnv")
                nc.scalar.activation(lnv[:], agg[:, 1:2], AF.Ln, bias=eps_col[:])
                rstd = smpool.tile([128, 1], FP32, tag="rstd")
                nc.scalar.activation(rstd[:], lnv[:], AF.Exp, scale=-0.5)
                hn_tm = spool.tile([128, D], BF16, tag="hn_tm", bufs=2)
                nc.vector.tensor_scalar(
                    out=hn_tm[:], in0=x0[:], scalar1=agg[:, 0:1],
                    scalar2=rstd[:], op0=ALU.subtract, op1=ALU.mult)
                # transpose to d-major
                ncols = min(128, EXT - c * 128)
                for k in range(DK):
                    ps_t = ppool.tile([128, 128], BF16, tag="p0", bufs=1, name="ps_t")
                    nc.tensor.transpose(ps_t[:], hn_tm[:, k * 128:(k + 1) * 128],
                                        ident[:])
                    nc.vector.tensor_copy(
                        out=h[:, k, c * 128:c * 128 + ncols],
                        in_=ps_t[:, :ncols])

            # ---------------- layers ----------------
            for l in range(n_layers):
                with nc.named_scope(f"L{l:02d}"):
                    h = layer(nc, l, h, wview, consts, ag_in, ag_out,
                              n_layers, pools)

            # ---------------- final mean + fc ----------------
            hsum = smpool.tile([128, DK], FP32, tag="hsum")
            for k in range(DK):
                nc.vector.tensor_reduce(out=hsum[:, k:k + 1],
                                        in_=h[:, k, W:W + OWN],
                                        axis=AX.X, op=ALU.add)
            ps_fc = ppool.tile([NCOUT, 1], FP32, tag="p0", bufs=1, name="ps_fc")
            for k in range(DK):
                nc.tensor.matmul(ps_fc[:], fcw[:, k, :], hsum[:, k:k + 1],
                                 start=(k == 0), stop=(k == DK - 1))
            out_sb = smpool.tile([NCOUT, 1], FP32, tag="out_sb")
            nc.vector.tensor_copy(out_sb[:], ps_fc[:])
            nc.sync.dma_start(out_d[:, :], out_sb[:])

    nc.compile()
    return nc


def ln_d_major(nc, src, dst, dst_off, tcs, consts, spool, smpool, ppool, tag):
    """Layernorm over d for d-major bf16 activations.

    src: [128, DK, ntok] AP; result written to dst[:, k, dst_off + t].
    tcs: list of (t_start, t_len) chunks (<=512).
    """
    ones_col, ones_row, cneg_row = consts[0], consts[1], consts[2]
    eps_row = consts[7][0:1, :]
    for (t0, tl) in tcs:
        sum_ps = ppool.tile([1, 512], FP32, tag="p2", bufs=1, name="sum_ps")
        sq_ps = ppool.tile([1, 512], FP32, tag="p3", bufs=1, name="sq_ps")
        for k in range(DK):
            sqt = spool.tile([128, 512], BF16, tag="sqt")
            nc.scalar.square(sqt[:, :tl], src[:, k, t0:t0 + tl])
            nc.tensor.matmul(sum_ps[:, :tl], ones_col[:], src[:, k, t0:t0 + tl],
                             start=(k == 0), stop=(k == DK - 1))
            nc.tensor.matmul(sq_ps[:, :tl], ones_col[:], sqt[:, :tl],
                             start=(k == 0), stop=(k == DK - 1))
        def row(nm):
            return smpool.tile([1, 512], FP32, tag="lnrow", bufs=5, name=nm)
        sum_sb = row("sum_sb")
        nc.vector.tensor_copy(sum_sb[:, :tl], sum_ps[:, :tl])
        t1 = row("t1")
        nc.vector.tensor_tensor(out=t1[:, :tl], in0=sum_sb[:, :tl],
                                in1=sum_sb[:, :tl], op=ALU.mult)
        t2 = row("t2")
        nc.vector.tensor_scalar(out=t2[:, :tl], in0=t1[:, :tl],
                                scalar1=-1.0 / D, scalar2=None, op0=ALU.mult)
        diff = row("diff")
        nc.vector.tensor_tensor(out=diff[:, :tl], in0=sq_ps[:, :tl],
                                in1=t2[:, :tl], op=ALU.add)
        dpos = row("dpos")
        nc.vector.tensor_scalar(out=dpos[:, :tl], in0=diff[:, :tl],
                                scalar1=0.0, scalar2=None, op0=ALU.max)
        lnv = row("lnv")
        nc.scalar.activation(lnv[:, :tl], dpos[:, :tl], AF.Ln,
                             bias=eps_row, scale=1.0 / D)
        rstd = row("rstd")
        nc.scalar.activation(rstd[:, :tl], lnv[:, :tl], AF.Exp, scale=-0.5)
        mr = row("mr")
        nc.vector.tensor_tensor(out=mr[:, :tl], in0=sum_sb[:, :tl],
                                in1=rstd[:, :tl], op=ALU.mult)
        r_bf = smpool.tile([1, 512], BF16, tag="lnrow_bf", bufs=2, name="r_bf")
        nc.vector.tensor_copy(r_bf[:, :tl], rstd[:, :tl])
        mr_bf = smpool.tile([1, 512], BF16, tag="lnrow_bf", bufs=2,
                            name="mr_bf")
        nc.vector.tensor_copy(mr_bf[:, :tl], mr[:, :tl])
        rb_ps = ppool.tile([128, 512], FP32, tag="p4", bufs=1, name="rb_ps")
        nc.tensor.matmul(rb_ps[:, :tl], ones_row[:], r_bf[:, :tl],
                         start=True, stop=True)
        mrb_ps = ppool.tile([128, 512], FP32, tag="p5", bufs=1, name="mrb_ps")
        nc.tensor.matmul(mrb_ps[:, :tl], cneg_row[:], mr_bf[:, :tl],
                         start=True, stop=True)
        rbs = spool.tile([128, 512], BF16, tag="rbs", bufs=2, name="rbs")
        nc.scalar.copy(rbs[:, :tl], rb_ps[:, :tl])
        mrbs = spool.tile([128, 512], BF16, tag="mrbs", bufs=2, name="mrbs")
        nc.scalar.copy(mrbs[:, :tl], mrb_ps[:, :tl])
        for k in range(DK):
            tmp = spool.tile([128, 512], BF16, tag="lnap")
            nc.vector.tensor_tensor(out=tmp[:, :tl], in0=src[:, k, t0:t0 + tl],
                                    in1=rbs[:, :tl], op=ALU.mult)
            nc.vector.tensor_tensor(out=dst[:, k, dst_off + t0:dst_off + t0 + tl],
                                    in0=tmp[:, :tl], in1=mrbs[:, :tl],
                                    op=ALU.add)


def layer(nc, l, h, wview, consts, ag_in, ag_out, n_layers, pools):
    hpool, bpool, spool, smpool, ppool = pools
    (ones_col, ones_row, cneg_row, ident, offs, hofs, maskP,
     eps_col) = consts
    TC_EXT = [(0, 512), (512, 512), (1024, EXT - 1024)]
    TC_OWN = [(0, 512), (512, 512)]

    # ---------------- QKV ----------------
    mmctr = [0]

    def mm_tile(name):
        t = ppool.tile([128, 512], FP32, tag=f"p{mmctr[0] % 2}", bufs=1,
                       name=name)
        mmctr[0] += 1
        return t

    with nc.named_scope(f"L{l:02d}_qkv"):
        q_sb = bpool.tile([128, DK, OWN], BF16, tag="q")
        k_sb = bpool.tile([128, DK, EXT], BF16, tag="k")
        for name, dst in (("wq", q_sb), ("wk", k_sb)):
            is_q = name == "wq"
            for ko in range(DK):
                wt = spool.tile([128, DK, 128], BF16, tag="wt")
                nc.sync.dma_start(wt[:], wview[name][l, :, :, ko * 128:(ko + 1) * 128])
                tcs = TC_OWN if is_q else TC_EXT
                off = W if is_q else 0
                for (t0, tl) in tcs:
                    ps = mm_tile("ps_qk")
                    for ki in range(DK):
                        nc.tensor.matmul(ps[:, :tl], wt[:, ki, :],
                                         h[:, ki, off + t0:off + t0 + tl],
                                         start=(ki == 0), stop=(ki == DK - 1))
                    nc.vector.tensor_copy(dst[:, ko, t0:t0 + tl], ps[:, :tl])
        # V: token-major [token, d]
        v_tm = bpool.tile([128, 9, D], BF16, tag="v")
        wv_t = bpool.tile([128, DK, D], BF16, tag="wv_full")
        nc.sync.dma_start(wv_t[:], wview["wv"][l])
        for c in range(9):
            ncols = min(128, EXT - c * 128)
            for d0, dl in ((0, 512), (512, 256)):
                ps = mm_tile("ps_v")
                for ki in range(DK):
                    nc.tensor.matmul(ps[:ncols, :dl],
                                     h[:, ki, c * 128:c * 128 + ncols],
                                     wv_t[:, ki, d0:d0 + dl],
                                     start=(ki == 0), stop=(ki == DK - 1))
                nc.scalar.copy(v_tm[:ncols, c, d0:d0 + dl], ps[:ncols, :dl])

    # ---------------- attention (head pairs) ----------------
    a_sb = bpool.tile([128, DK, OWN], BF16, tag="attn")
    attn_scope = nc.named_scope(f"L{l:02d}_attn")
    attn_scope.__enter__()
    for qc in range(NQC):
        for j in range(H // 2):
            par = (qc * (H // 2) + j) % 3
            g0 = qc * 128
            # one PSUM bank holds both heads' scores: [evA|odA|evB|odB]
            sc = ppool.tile([128, 512], FP32, tag=f"p{2 + 2 * par}", bufs=1,
                            name="sc")
            nc.tensor.matmul(sc[:], ident[:], maskP[:, qc, :],
                             start=True, stop=False)
            for par_h, ro in ((0, 0), (1, 64)):
                q_ap = q_sb[ro:ro + 64, j, g0:g0 + 128]
                nc.tensor.matmul(sc[:, 128 * par_h:128 * par_h + 128],
                                 k_sb[ro:ro + 64, j, g0:g0 + 128], q_ap,
                                 start=False, stop=False, skip_group_check=True)
                nc.tensor.matmul(sc[:64, 256 + 128 * par_h:384 + 128 * par_h],
                                 k_sb[ro:ro + 64, j, g0 + 128:g0 + GW], q_ap,
                                 start=False, stop=(par_h == 1),
                                 skip_group_check=True)
            eA = spool.tile([128, 256], BF16, tag="eA")
            nc.scalar.activation(eA[:], sc[:, 0:256], AF.Exp)
            eB = spool.tile([64, 256], BF16, tag="eB")
            nc.scalar.activation(eB[:], sc[:64, 256:512], AF.Exp)
            pvse = ppool.tile([128, 512], FP32, tag=f"p{3 + 2 * par}", bufs=1,
                              name="pvse")
            se = pvse[0:1, 256:512]
            nc.tensor.matmul(se, ones_col[:], eA[:], start=True, stop=False)
            nc.tensor.matmul(se, ones_col[:64, :], eB[:], start=False,
                             stop=True)
            for par_h, po in ((0, 0), (1, 64)):
                pv = pvse[po:po + 64, 128 * par_h:128 * par_h + 128]
                nc.tensor.matmul(
                    pv, v_tm[:, qc, 128 * j + 64 * par_h:128 * j + 64 * par_h + 64],
                    eA[:, 128 * par_h:128 * par_h + 128], start=True, stop=False,
                    tile_position=(0, po), skip_group_check=True)
                nc.tensor.matmul(
                    pv, v_tm[:64, qc + 1, 128 * j + 64 * par_h:128 * j + 64 * par_h + 64],
                    eB[:, 128 * par_h:128 * par_h + 128], start=False, stop=True,
                    tile_position=(0, po), skip_group_check=True)
            rcp_bf = smpool.tile([1, 256], BF16, tag="rcp_bf")
            with nc.allow_low_precision("softmax denominator"):
                nc.vector.reciprocal(rcp_bf[:], se)
            rb = pvse[:, 256:512]
            nc.tensor.matmul(rb, ones_row[:], rcp_bf[:], start=True,
                             stop=True, skip_group_check=True)
            rb_sb = spool.tile([128, 256], BF16, tag="rb_sb")
            nc.vector.tensor_copy(rb_sb[:], rb)
            for par_h, po in ((0, 0), (1, 64)):
                nc.vector.tensor_tensor(
                    out=a_sb[po:po + 64, j, g0:g0 + 128],
                    in0=pvse[po:po + 64, 128 * par_h:128 * par_h + 128],
                    in1=rb_sb[po:po + 64, 128 * par_h:128 * par_h + 128],
                    op=ALU.mult)

    attn_scope.__exit__(None, None, None)

    # ---------------- Wo + residual -> LN1 -> h2 ----------------
    wo_scope = nc.named_scope(f"L{l:02d}_wo_ln1")
    wo_scope.__enter__()
    h2raw = bpool.tile([128, DK, OWN], BF16, tag="h2raw")
    for ko in range(DK):
        wt = spool.tile([128, DK, 128], BF16, tag="wt")
        nc.sync.dma_start(wt[:], wview["wo"][l, :, :, ko * 128:(ko + 1) * 128])
        for (t0, tl) in TC_OWN:
            ps = mm_tile("ps_wo")
            for ki in range(DK):
                nc.tensor.matmul(ps[:, :tl], wt[:, ki, :], a_sb[:, ki, t0:t0 + tl],
                                 start=(ki == 0), stop=(ki == DK - 1))
            nc.vector.tensor_tensor(out=h2raw[:, ko, t0:t0 + tl],
                                    in0=h[:, ko, W + t0:W + t0 + tl],
                                    in1=ps[:, :tl], op=ALU.add)
    h2 = bpool.tile([128, DK, OWN], BF16, tag="h2")
    ln_d_major(nc, h2raw, h2, 0, TC_OWN, consts, spool, smpool, ppool, "ln1")
    wo_scope.__exit__(None, None, None)

    # ---------------- FFN -> residual -> LN2 -> h3 ----------------
    ffn_scope = nc.named_scope(f"L{l:02d}_ffn")
    ffn_scope.__enter__()
    h3raw = bpool.tile([128, DK, OWN], BF16, tag="h3raw")
    for (t0, tl) in TC_OWN:
        accs = [ppool.tile([128, 512], FP32, tag=f"p{j + 2}", bufs=1,
                           name=f"acc{j}") for j in range(DK)]
        for j in range(DFFK):
            w1t = spool.tile([128, DK, 128], BF16, tag="w1t")
            nc.sync.dma_start(w1t[:], wview["w1"][l, :, :, j * 128:(j + 1) * 128])
            w2t = spool.tile([128, D], BF16, tag="w2t")
            nc.sync.dma_start(w2t[:], wview["w2"][l, :, j, :])
            ps1 = ppool.tile([128, 512], FP32, tag=f"p{j % 2}", bufs=1,
                             name="ps1")
            for ki in range(DK):
                nc.tensor.matmul(ps1[:, :tl], w1t[:, ki, :], h2[:, ki, t0:t0 + tl],
                                 start=(ki == 0), stop=(ki == DK - 1))
            g = spool.tile([128, 512], BF16, tag="gel")
            nc.scalar.activation(g[:, :tl], ps1[:, :tl], AF.Gelu)
            for ko in range(DK):
                nc.tensor.matmul(accs[ko][:, :tl], w2t[:, ko * 128:(ko + 1) * 128],
                                 g[:, :tl], start=(j == 0), stop=(j == DFFK - 1))
        for ko in range(DK):
            nc.vector.tensor_tensor(out=h3raw[:, ko, t0:t0 + tl],
                                    in0=h2[:, ko, t0:t0 + tl],
                                    in1=accs[ko][:, :tl], op=ALU.add)
    h3 = hpool.tile([128, DK, EXT], BF16, tag="h")
    ln_d_major(nc, h3raw, h3, W, TC_OWN, consts, spool, smpool, ppool, "ln2")
    ffn_scope.__exit__(None, None, None)

    # ---------------- halo exchange ----------------
    if l < n_layers - 1:
        with nc.named_scope(f"L{l:02d}_halo"):
            agi = ag_in[l].ap().rearrange("(a p) s c -> p a s c", p=128)
            nc.sync.dma_start(agi[:, :, 0, :], h3[:, :, W:2 * W])
            nc.sync.dma_start(agi[:, :, 1, :], h3[:, :, OWN:W + OWN])
            nc.gpsimd.collective_compute(
                "AllGather", ALU.bypass,
                replica_groups=[list(range(NC_CORES))],
                ins=[ag_in[l].ap()], outs=[ag_out[l].ap()],
            )
            agv = ag_out[l].ap().rearrange("s d b c -> (s d b) c")
            for k in range(DK):
                nc.gpsimd.indirect_dma_start(
                    out=h3[:, k, 0:W], out_offset=None, in_=agv[:, :],
                    in_offset=IndirectOffsetOnAxis(ap=hofs[:, k:k + 1], axis=0),
                )
                nc.gpsimd.indirect_dma_start(
                    out=h3[:, k, W + OWN:EXT], out_offset=None, in_=agv[:, :],
                    in_offset=IndirectOffsetOnAxis(
                        ap=hofs[:, DK + k:DK + k + 1], axis=0),
                )
    return h3


_NC_CACHE = {}


def _get_nc(n_layers):
    if n_layers not in _NC_CACHE:
        _NC_CACHE[n_layers] = build_nc(n_layers)
    return _NC_CACHE[n_layers]


def make_in_maps(x, emb, pos_emb, tok_emb, Wq, Wk, Wv, Wo, W1, W2, fc_w,
                 n_layers):
    x = np.asarray(x)
    bf = lambda a: np.ascontiguousarray(np.asarray(a), dtype=np.float32).astype(
        ml_dtypes.bfloat16)
    scale = 1.0 / np.sqrt(np.float32(DH))
    shared = {
        "emb": bf(emb),
        "wq": bf(np.asarray(Wq)[:n_layers] * scale),
        "wk": bf(np.asarray(Wk)[:n_layers]),
        "wv": bf(np.asarray(Wv)[:n_layers]),
        "wo": bf(np.asarray(Wo)[:n_layers]),
        "w1": bf(np.asarray(W1)[:n_layers]),
        "w2": bf(np.asarray(W2)[:n_layers]),
        "fcw": np.ascontiguousarray(np.asarray(fc_w), dtype=np.float32),
        "ident": np.eye(128, dtype=ml_dtypes.bfloat16),
    }
    postok_full = (np.asarray(pos_emb)[1:S + 1] + np.asarray(tok_emb)[0]
                   ).astype(np.float32)

    in_maps = []
    for c in range(NC_CORES):
        b, s_idx = divmod(c, SHARDS_PER_B)
        own0 = s_idx * OWN
        ext_pos = np.arange(own0 - W, own0 - W + EXTP)
        valid = (ext_pos >= 0) & (ext_pos < S)
        pos_c = np.clip(ext_pos, 0, S - 1)
        xids = np.where(valid, x[b][pos_c], 1).astype(np.int32)
        postok = np.where(valid[:, None], postok_full[pos_c], 0.0
                          ).astype(np.float32)
        # masks: scores[g, t] for query chunk qc; key global position is
        # own0 - W + qc*128 + g, query global position own0 + qc*128 + t.
        gi = np.arange(GW)[:, None]
        ti = np.arange(128)[None, :]
        band = np.abs((gi - W) - ti) <= W
        mP = np.zeros((NQC, 128, 512), ml_dtypes.bfloat16)
        for qc in range(NQC):
            kpos = own0 - W + qc * 128 + np.arange(GW)
            ok = band & ((kpos >= 0) & (kpos < S))[:, None]
            m = np.where(ok, 0.0, MASK_NEG).astype(ml_dtypes.bfloat16)
            mP[qc, :, 0:128] = m[:128]
            mP[qc, :, 128:256] = m[:128]
            mP[qc, :64, 256:384] = m[128:]
            mP[qc, :64, 384:512] = m[128:]
        # halo gather offsets into ag_out viewed as rows [(8*768*2), 32]
        slot_l, slot_r = max(c - 1, 0), min(c + 1, NC_CORES - 1)
        p = np.arange(128)
        hofs = np.empty((2 * DK, 128), np.int32)
        for k in range(DK):
            hofs[k] = (slot_l * D + k * 128 + p) * 2 + 1
            hofs[DK + k] = (slot_r * D + k * 128 + p) * 2 + 0
        in_maps.append({
            **shared, "xids": xids.reshape(EXTP // 128, 128),
            "postok": postok, "maskP": mP, "hofs": hofs,
        })
    return in_maps


def kernel(x, emb, pos_emb, tok_emb, emb_ln_s, emb_ln_b, Wq, bq, Wk, bk,
           Wv, bv, Wo, bo, ln1_s, ln1_b, W1, b1, W2, b2, ln2_s, ln2_b,
           fc_w, fc_b, _n_layers=None, _results_hook=None):
    n_layers = _n_layers if _n_layers is not None else L
    for z in (bq, bk, bv, bo, b1, b2, emb_ln_b, ln1_b, ln2_b):
        assert not np.any(np.asarray(z)), "nonzero biases unsupported"
    for o in (emb_ln_s, ln1_s, ln2_s):
        assert np.all(np.asarray(o) == 1.0), "non-unit LN scales unsupported"

    in_maps = make_in_maps(x, emb, pos_emb, tok_emb, Wq, Wk, Wv, Wo, W1, W2,
                           fc_w, n_layers)
    nc = _get_nc(n_layers)
    res = run_bass_kernel_spmd(nc, in_maps, list(range(NC_CORES)))
    if _results_hook is not None:
        _results_hook(res)
    out = np.zeros((B, NCOUT), np.float32)
    for c in range(NC_CORES):
        out[c // SHARDS_PER_B] += res.results[c]["out"][:, 0]
    out = out / np.float32(S) + np.asarray(fc_b, np.float32)
    return out

